# revision 1
# baseline (speedup 1.0000x reference)
"""LIF spiking-neuron layer on 8 Trainium2 NeuronCores (Bass/Tile).

Reference semantics (per neuron, T=6 steps, v0=0):
    v = v*0.5 + x_t ; s = (v >= 1.0) ; v = v - s
Output: spikes [T, B, C, H, W] float32 (values are exactly 0.0 / 1.0).

Sharding: data-parallel over batch (axis 1): 64 batches / 8 cores.
Per core the neuron field (8*128*32*32 = 1,048,576 elements) is laid
out as [128 partitions, 8192 cols], processed in 4 column blocks of
2048 with a 6-step sequential recurrence per block.

Per-core compute (bit-identical to the fp32 reference):
  state kept as h = v/2 (exact power-of-2 scale).
  u_t  = h_{t-1} + x_t       <- performed by the load DMA itself
                                (SWDGE accum_op=add, fp32)
  sh_t = (u_t >= 1) * 0.5    <- one DVE tensor_scalar (dual-op), fp8e4
                                out; {0, 0.5} both exact in fp8e4
  h_t  = (u_t * 0.5) - sh_t  <- one DVE scalar_tensor_tensor, in place
Spikes are stored as fp8e4 {0, 0.5} (1/4 HBM store traffic vs f32);
the host multiplies by 2 -> exact {0,1} float32.

Engine budget per core (cost model): ~2 DVE ops/step (~82-99us busy),
HBM traffic 30 MiB (~85us) -> modeled e2e ~98us, vs ~146us for the
naive fp32-store 3-op-per-step version (all verified bit-exact on HW).
"""

import os
import sys

import numpy as np

sys.path.insert(0, "/opt/trn_rl_repo")

import concourse.bacc as bacc
import concourse.bass as bass
import concourse.mybir as mybir
from concourse import tile
from concourse.bass_utils import run_bass_kernel_spmd

T = 6
B = 64
C = 128
H = 32
W = 32
N_CORES = 8
B_PER_CORE = B // N_CORES
N_PER_CORE = B_PER_CORE * C * H * W  # 1,048,576
P = 128
FTOT = N_PER_CORE // P               # 8192
FBLK = 2048
NBLK = FTOT // FBLK                  # 4

_COMPILED = None
LAST_RESULTS = None


def _build_program():
    nc = bacc.Bacc(None, target_bir_lowering=False, debug=False)

    f32, f8 = mybir.dt.float32, mybir.dt.float8e4
    x_d = nc.dram_tensor("x", [T, N_PER_CORE], f32, kind="ExternalInput")
    s_d = nc.dram_tensor("s", [T, N_PER_CORE], f8, kind="ExternalOutput")
    x_r = x_d[:].rearrange("t (p f) -> t p f", p=P)

    with tile.TileContext(nc) as tc:
        with (
            tc.tile_pool(name="u", bufs=NBLK) as u_pool,
            tc.tile_pool(name="s6", bufs=NBLK) as s_pool,
        ):
            for blk in range(NBLK):
                c0 = blk * FBLK
                u = u_pool.tile([P, FBLK], f32, tag="u")
                # u_0 = x_0 (v0 = 0); plain HWDGE load
                nc.sync.dma_start(out=u[:], in_=x_r[0][:, c0:c0 + FBLK])
                s6 = s_pool.tile([P, T * FBLK], f8, tag="s6")
                for t in range(T):
                    sl = s6[:, t * FBLK:(t + 1) * FBLK]
                    # sh = (u >= 1.0) * 0.5 -> fp8e4 {0, 0.5}, both exact
                    nc.vector.tensor_scalar(
                        out=sl, in0=u[:], scalar1=1.0, scalar2=0.5,
                        op0=mybir.AluOpType.is_ge,
                        op1=mybir.AluOpType.mult,
                    )
                    if t < T - 1:
                        # h = (u * 0.5) - sh, in place
                        nc.vector.scalar_tensor_tensor(
                            out=u[:], in0=u[:], scalar=0.5, in1=sl,
                            op0=mybir.AluOpType.mult,
                            op1=mybir.AluOpType.subtract,
                        )
                        # u_{t+1} = h + x_{t+1}: accumulate during load
                        nc.gpsimd.dma_start(
                            out=u[:], in_=x_r[t + 1][:, c0:c0 + FBLK],
                            accum_op=mybir.AluOpType.add,
                        )
                    # store spikes as soon as a pair of timesteps is done
                    if (t + 1) % 2 == 0:
                        tlo = t - 1
                        sb = s6[:, tlo * FBLK:(t + 1) * FBLK].rearrange(
                            "p (t f) -> p t f", t=2)
                        dram_ap = bass.AP(
                            s_d, tlo * N_PER_CORE + c0,
                            [[FTOT, P], [N_PER_CORE, 2], [1, FBLK]])
                        nc.sync.dma_start(out=dram_ap, in_=sb)
    nc.finalize()
    return nc


def kernel(input_current: np.ndarray) -> np.ndarray:
    global _COMPILED, LAST_RESULTS
    x = np.asarray(input_current, dtype=np.float32)
    assert x.shape == (T, B, C, H, W), x.shape

    if _COMPILED is None:
        _COMPILED = _build_program()
    nc = _COMPILED

    in_maps = []
    for k in range(N_CORES):
        shard = np.ascontiguousarray(
            x[:, k * B_PER_CORE:(k + 1) * B_PER_CORE]
        ).reshape(T, N_PER_CORE)
        in_maps.append({"x": shard})

    trace = bool(int(os.environ.get("LIF_TRACE", "0")))
    res = run_bass_kernel_spmd(nc, in_maps, core_ids=list(range(N_CORES)),
                               trace=trace)
    LAST_RESULTS = res

    out = np.empty((T, B, C, H, W), dtype=np.float32)
    for k in range(N_CORES):
        sh = res.results[k]["s"].astype(np.float32) * 2.0
        out[:, k * B_PER_CORE:(k + 1) * B_PER_CORE] = (
            sh.reshape(T, B_PER_CORE, C, H, W)
        )
    return out



# revision 17
# speedup vs baseline: 1.0667x; 1.0667x over previous
"""LIF spiking-neuron layer on 8 Trainium2 NeuronCores (Bass/Tile).

Reference semantics (per neuron, T=6 steps, v0=0):
    v = v*0.5 + x_t ; s = (v >= 1.0) ; v = v - s
Output: spikes [T, B, C, H, W] float32 (values are exactly 0.0 / 1.0).

Sharding: data-parallel over batch (axis 1): 64 batches / 8 cores.
Per core the neuron field (8*128*32*32 = 1,048,576 elements) is laid
out as [128 partitions, 8192 cols], processed in 4 column blocks of
2048 with a 6-step sequential recurrence per block (blocks skewed by
one timestep so engines round-robin between them).

Scaled-state formulation (bit-identical to the fp32 reference):
  state U_t = 2^t * v_t; host pre-scales inputs xs_t = 2^t * x_t
  (exact power-of-2 scalings commute with fp32 round-to-nearest).
  Per step:
    w_t = (U_t >= 2^t) * 2^t    DVE tensor_scalar (2x mode), fp8e4
                                ({0, 2^t} exact in fp8e4)
    U  -=  w_t                  scalar_tensor_tensor (DVE)
    U  +=  xs_{t+1}             scalar_tensor_tensor (GPSIMD/Pool) from a
                                staging tile prefetched by plain HWDGE DMA
  The idle TensorEngine packs the 6 spike planes into a 6-bit code
  C = sum_t 2^t s_t via identity matmuls accumulated in PSUM (exact
  small-integer arithmetic), ScalarE casts PSUM->SBUF uint8, and the
  store writes 1 MiB/core instead of 6 MiB -> HBM traffic drops from
  ~30 MiB to ~25.2 MiB per core.  The host unpacks bits to {0,1} f32.
"""

import os
import sys

import numpy as np

sys.path.insert(0, "/opt/trn_rl_repo")

import concourse.bacc as bacc
import concourse.bass as bass
import concourse.mybir as mybir
from concourse import tile
from concourse.bass_utils import run_bass_kernel_spmd
from concourse.masks import make_identity

T = 6
B = 64
C = 128
H = 32
W = 32
N_CORES = 8
B_PER_CORE = B // N_CORES
N_PER_CORE = B_PER_CORE * C * H * W  # 1,048,576
P = 128
FTOT = N_PER_CORE // P               # 8192
if os.environ.get("LIF_BLOCKS"):
    BLOCKS = [int(v) for v in os.environ["LIF_BLOCKS"].split(",")]
    assert sum(BLOCKS) == FTOT, BLOCKS
else:
    _n = int(os.environ.get("LIF_NBLK", "4"))
    BLOCKS = [FTOT // _n] * _n
NBLK = len(BLOCKS)
OFFS = [sum(BLOCKS[:i]) for i in range(NBLK)]
MM = 512                             # PE moving-free / PSUM window
SKEW = int(os.environ.get("LIF_SKEW", "1"))
REV = int(os.environ.get("LIF_REV", "0"))
PSUM_BUFS = int(os.environ.get("LIF_PSUM_BUFS", "2"))

# (blk, t) state subtracts run on GPSIMD instead of DVE (load balance)
_GP_MODE = os.environ.get("LIF_GP", "checker")
if _GP_MODE == "none":
    GPSIMD_STT = set()
elif _GP_MODE == "checker":
    GPSIMD_STT = {(blk, t) for blk in range(NBLK) for t in range(T - 1)
                  if (blk + t) % 2 == 1}
elif _GP_MODE == "bblock":
    GPSIMD_STT = {(blk, t) for blk in range(NBLK) for t in range(T - 1)
                  if blk % 2 == 1}
else:  # "k=<n>": first n in (blk major) order
    _k = int(_GP_MODE.split("=")[1])
    _all = [(blk, t) for t in range(T - 1) for blk in range(NBLK)]
    GPSIMD_STT = set(_all[:_k])

_COMPILED = None
LAST_RESULTS = None


def _build_program():
    nc = bacc.Bacc(None, target_bir_lowering=False, debug=False)

    f32 = mybir.dt.float32
    f8 = mybir.dt.float8e4
    u8 = mybir.dt.uint8
    A = mybir.AluOpType

    x_d = nc.dram_tensor("x", [T, N_PER_CORE], f32, kind="ExternalInput")
    c_d = nc.dram_tensor("c", [N_PER_CORE], u8, kind="ExternalOutput")
    x_r = x_d[:].rearrange("t (p f) -> t p f", p=P)
    c_r = c_d[:].rearrange("(p f) -> p f", p=P)

    with tile.TileContext(nc) as tc:
        with (
            tc.tile_pool(name="consts", bufs=1) as consts,
            tc.tile_pool(name="u", bufs=1) as u_pool,
            tc.tile_pool(name="w6", bufs=1) as w_pool,
            tc.tile_pool(name="cp", bufs=int(os.environ.get("LIF_CP_BUFS", "1")), space="PSUM") as cp_pool,
            tc.tile_pool(name="cpl", bufs=1, space="PSUM") as cpl_pool,
            tc.tile_pool(name="cs", bufs=2) as cs_pool,
        ):
            ident = consts.tile([P, P], f8, name="ident")
            make_identity(nc, ident)

            u = [None] * NBLK
            w6 = [None] * NBLK
            cp_last = [None]

            def finish_block(blk, cp):
                c0, fb = OFFS[blk], BLOCKS[blk]
                cs = cs_pool.tile([P, fb], u8, tag=f"cs{fb}",
                                  name=f"cs{blk}")
                nc.scalar.copy(out=cs[:], in_=cp[:])  # PSUM->SBUF, u8
                nc.sync.dma_start(out=c_r[:, c0:c0 + fb], in_=cs[:])

            def author_step(blk, t):
                c0, fb = OFFS[blk], BLOCKS[blk]
                thr = float(2.0 ** t)
                last_blk = blk == NBLK - 1
                if t == 0:
                    u[blk] = u_pool.tile([P, fb], f32, tag=f"u{blk}",
                                         name=f"u{blk}")
                    # U_0 = xs_0 (v0 = 0); plain HWDGE load
                    nc.sync.dma_start(out=u[blk][:],
                                      in_=x_r[0][:, c0:c0 + fb])
                    w6[blk] = w_pool.tile([P, T * fb], f8, tag=f"w6b{blk}",
                                          name=f"w6_{blk}")
                sl = w6[blk][:, t * fb:(t + 1) * fb]
                # w = (U >= 2^t) * 2^t  -> fp8e4 {0, 2^t}, both exact
                nc.vector.tensor_scalar(
                    out=sl, in0=u[blk][:], scalar1=thr, scalar2=thr,
                    op0=A.is_ge, op1=A.mult,
                )
                if t < T - 1:
                    # U = U - w, in place (tensor_tensor: valid on both
                    # DVE and Pool ISAs; scalar_tensor_tensor is DVE-only)
                    eng = (nc.gpsimd if (blk, t) in GPSIMD_STT
                           else nc.vector)
                    eng.tensor_tensor(
                        out=u[blk][:], in0=u[blk][:], in1=sl,
                        op=A.subtract,
                    )
                    # U += xs_{t+1}: accumulate during load (SWDGE CCE add)
                    nc.gpsimd.dma_start(
                        out=u[blk][:], in_=x_r[t + 1][:, c0:c0 + fb],
                        accum_op=A.add,
                    )
                if last_blk:
                    # last block: pack per step into a dedicated PSUM
                    # accumulator so only t=5's matmuls are in the tail
                    if t == 0:
                        cp_last[0] = cpl_pool.tile([P, fb], f32,
                                                   tag="cpl", name="cpl")
                    for j0 in range(0, fb, MM):
                        m = min(MM, fb - j0)
                        o = t * fb + j0
                        nc.tensor.matmul(
                            cp_last[0][:, j0:j0 + m],
                            ident[:], w6[blk][:, o:o + m],
                            start=(t == 0), stop=(t == T - 1),
                        )
                    if t == T - 1:
                        finish_block(blk, cp_last[0])
                elif t == T - 1:
                    # other blocks: end-of-block PE burst
                    maxfb = max(BLOCKS[:-1])
                    cp_full = cp_pool.tile([P, maxfb], f32, tag="cp",
                                           name=f"cp{blk}")
                    cp = cp_full[:, :fb]
                    for j0 in range(0, fb, MM):
                        m = min(MM, fb - j0)
                        for tt in range(T):
                            o = tt * fb + j0
                            nc.tensor.matmul(
                                cp[:, j0:j0 + m],
                                ident[:],
                                w6[blk][:, o:o + m],
                                start=(tt == 0), stop=(tt == T - 1),
                            )
                    finish_block(blk, cp)

            # skewed rounds: block b runs step (r - SKEW*b)
            order = list(range(NBLK))
            if REV:
                order = order[::-1]
            for r in range(T + SKEW * (NBLK - 1) if SKEW else T * NBLK):
                if SKEW:
                    for blk in order:
                        t = r - SKEW * blk
                        if 0 <= t < T:
                            author_step(blk, t)
                else:
                    author_step(r % NBLK if False else r // T, r % T)
    nc.finalize()
    return nc


_XS_SCALE = (2.0 ** np.arange(T, dtype=np.float32)).reshape(T, 1)


def kernel(input_current: np.ndarray) -> np.ndarray:
    global _COMPILED, LAST_RESULTS
    x = np.asarray(input_current, dtype=np.float32)
    assert x.shape == (T, B, C, H, W), x.shape

    if _COMPILED is None:
        _COMPILED = _build_program()
    nc = _COMPILED

    in_maps = []
    for k in range(N_CORES):
        shard = np.ascontiguousarray(
            x[:, k * B_PER_CORE:(k + 1) * B_PER_CORE]
        ).reshape(T, N_PER_CORE)
        # xs_t = 2^t * x_t (exact in fp32: pure exponent shift)
        in_maps.append({"x": shard * _XS_SCALE})

    trace = bool(int(os.environ.get("LIF_TRACE", "0")))
    res = run_bass_kernel_spmd(nc, in_maps, core_ids=list(range(N_CORES)),
                               trace=trace)
    LAST_RESULTS = res

    out = np.empty((T, B, C, H, W), dtype=np.float32)
    bits = np.arange(T, dtype=np.uint8).reshape(T, 1)
    for k in range(N_CORES):
        code = res.results[k]["c"]  # uint8 [N_PER_CORE], values 0..63
        sp = ((code[None, :] >> bits) & 1).astype(np.float32)
        out[:, k * B_PER_CORE:(k + 1) * B_PER_CORE] = (
            sp.reshape(T, B_PER_CORE, C, H, W)
        )
    return out


# revision 29
# speedup vs baseline: 1.0931x; 1.0248x over previous
"""LIF spiking-neuron layer on 8 Trainium2 NeuronCores (Bass/Tile).

Reference semantics (per neuron, T=6 steps, v0=0):
    v = v*0.5 + x_t ; s = (v >= 1.0) ; v = v - s
Output: spikes [T, B, C, H, W] float32 (values are exactly 0.0 / 1.0).

Sharding: data-parallel over batch (axis 1): 64 batches / 8 cores.
Per core the neuron field (8*128*32*32 = 1,048,576 elements) is laid
out as [128 partitions, 8192 cols], processed in 8 column blocks of
descending size (1536..512).  Blocks are software-pipelined with a
one-timestep skew (authored shallowest-first) so that while one block
waits on its input DMA the engines serve the other blocks; descending
sizes fill the pipeline fast at the start and drain it fast at the end.

Scaled-state formulation (bit-identical to the fp32 reference):
  state U_t = 2^t * v_t; host pre-scales inputs xs_t = 2^t * x_t
  (exact power-of-2 scalings commute with fp32 round-to-nearest, so
  every rounding matches the reference bit-for-bit).
  Per step:
    w_t = (U_t >= 2^t) * 2^t    DVE tensor_scalar (2x perf mode), fp8e4
                                plane ({0, 2^t} exact in fp8e4)
    U  -=  w_t                  tensor_tensor subtract (DVE; the t=0 one
                                runs on GPSIMD/Pool - it only depends on
                                the initial load so it never head-blocks
                                the Pool queue, and it relieves the DVE
                                during pipeline fill)
    U  +=  xs_{t+1}             performed by the load DMA itself
                                (SWDGE accum_op=add, fp32 - zero engine
                                cost, rides the DMA device)
  The otherwise-idle TensorEngine packs the 6 fp8 spike planes into a
  6-bit code C = sum_t 2^t s_t via identity matmuls accumulated in PSUM
  (exact small-integer arithmetic; the last block accumulates per-step
  so only one matmul remains in the tail), ScalarE copies PSUM->SBUF
  with a uint8 cast, and the store writes 1 MiB/core instead of the
  6 MiB an fp8-per-timestep output needs -> HBM traffic drops from
  ~30 MiB to ~25.2 MiB per core.  The host unpacks bits to {0,1} f32.

Cost-model timeline: ~89.5 us vs 97.8 us for the previous kernel; DMA
busy is 72.8 us (25.2 MiB at the 360 GB/s model rate), DVE ~63 us.
"""

import os
import sys

import numpy as np

sys.path.insert(0, "/opt/trn_rl_repo")

import concourse.bacc as bacc
import concourse.bass as bass
import concourse.mybir as mybir
from concourse import tile
from concourse.bass_utils import run_bass_kernel_spmd
from concourse.masks import make_identity

T = 6
B = 64
C = 128
H = 32
W = 32
N_CORES = 8
B_PER_CORE = B // N_CORES
N_PER_CORE = B_PER_CORE * C * H * W  # 1,048,576
P = 128
FTOT = N_PER_CORE // P               # 8192
if os.environ.get("LIF_BLOCKS"):
    BLOCKS = [int(v) for v in os.environ["LIF_BLOCKS"].split(",")]
    assert sum(BLOCKS) == FTOT, BLOCKS
elif os.environ.get("LIF_NBLK"):
    _n = int(os.environ["LIF_NBLK"])
    BLOCKS = [FTOT // _n] * _n
else:
    # descending sizes: big blocks fill the pipeline early, small tail
    # blocks drain it quickly
    BLOCKS = [1536, 1280, 1024, 1024, 1024, 1024, 768, 512]
NBLK = len(BLOCKS)
OFFS = [sum(BLOCKS[:i]) for i in range(NBLK)]
MM = 512                             # PE moving-free / PSUM window
SKEW = int(os.environ.get("LIF_SKEW", "1"))
REV = int(os.environ.get("LIF_REV", "1"))
PSUM_BUFS = int(os.environ.get("LIF_PSUM_BUFS", "2"))

# (blk, t) state subtracts run on GPSIMD instead of DVE (load balance)
_GP_MODE = os.environ.get("LIF_GP", "t0")
if _GP_MODE == "t0":
    # offload the t=0 state subtract of every block to GPSIMD: it only
    # depends on the initial load, so it never head-blocks the Pool
    # queue, and it relieves the DVE during pipeline fill
    GPSIMD_STT = {(blk, 0) for blk in range(NBLK)}
elif _GP_MODE == "none":
    GPSIMD_STT = set()
elif _GP_MODE == "checker":
    GPSIMD_STT = {(blk, t) for blk in range(NBLK) for t in range(T - 1)
                  if (blk + t) % 2 == 1}
elif _GP_MODE == "bblock":
    GPSIMD_STT = {(blk, t) for blk in range(NBLK) for t in range(T - 1)
                  if blk % 2 == 1}
elif _GP_MODE.startswith("list:"):
    GPSIMD_STT = {tuple(int(v) for v in it.split(","))
                  for it in _GP_MODE[5:].split(";") if it}
else:  # "k=<n>": first n in (blk major) order
    _k = int(_GP_MODE.split("=")[1])
    _all = [(blk, t) for t in range(T - 1) for blk in range(NBLK)]
    GPSIMD_STT = set(_all[:_k])

T5ACT = bool(int(os.environ.get("LIF_T5ACT", "0")))

_COMPILED = None
LAST_RESULTS = None


def _build_program():
    nc = bacc.Bacc(None, target_bir_lowering=False, debug=False)

    f32 = mybir.dt.float32
    f8 = mybir.dt.float8e4
    u8 = mybir.dt.uint8
    A = mybir.AluOpType

    x_d = nc.dram_tensor("x", [T, N_PER_CORE], f32, kind="ExternalInput")
    c_d = nc.dram_tensor("c", [N_PER_CORE], u8, kind="ExternalOutput")
    x_r = x_d[:].rearrange("t (p f) -> t p f", p=P)
    c_r = c_d[:].rearrange("(p f) -> p f", p=P)

    with tile.TileContext(nc) as tc:
        with (
            tc.tile_pool(name="consts", bufs=1) as consts,
            tc.tile_pool(name="u", bufs=1) as u_pool,
            tc.tile_pool(name="w6", bufs=1) as w_pool,
            tc.tile_pool(name="cp", bufs=int(os.environ.get("LIF_CP_BUFS", "2")),
                         space="PSUM") as cp_pool,
            tc.tile_pool(name="cpl",
                         bufs=int(os.environ.get("LIF_NLAST", "1")),
                         space="PSUM") as cpl_pool,
            tc.tile_pool(name="cs", bufs=int(os.environ.get("LIF_CS_BUFS", "2"))) as cs_pool,
        ):
            ident = consts.tile([P, P], f8, name="ident")
            make_identity(nc, ident)
            t5act = bool(int(os.environ.get("LIF_T5ACT", "0")))
            t5pool = bool(int(os.environ.get("LIF_T5POOL", "0")))
            if t5act or t5pool:
                # 32*I stationary + all-ones moving plane for the t=5
                # sign-based pack: contribution 32*sigma + 32 in {0,32,64}
                ident32 = consts.tile([P, P], f8, name="ident32")
                nc.gpsimd.memset(ident32[:], 0.0)
                nc.gpsimd.affine_select(
                    out=ident32[:], in_=ident32[:],
                    compare_op=mybir.AluOpType.not_equal, fill=32.0,
                    base=0, pattern=[[-1, P]], channel_multiplier=1,
                )
                ones = consts.tile([P, max(BLOCKS)], f8, name="ones")
                nc.gpsimd.memset(ones[:], 1.0)
                biasm32 = consts.tile([P, 1], f32, name="biasm32")
                nc.gpsimd.memset(biasm32[:], -32.0)
                thr32 = consts.tile([P, 1], f32, name="thr32")
                nc.gpsimd.memset(thr32[:], 32.0)

            u = [None] * NBLK
            w6 = [None] * NBLK
            cp_last = {}
            nlast = int(os.environ.get("LIF_NLAST", "1"))

            def finish_block(blk, cp):
                c0, fb = OFFS[blk], BLOCKS[blk]
                cs = cs_pool.tile([P, fb], u8, tag=f"cs{fb}",
                                  name=f"cs{blk}")
                nc.scalar.copy(out=cs[:], in_=cp[:])  # PSUM->SBUF, u8
                nc.sync.dma_start(out=c_r[:, c0:c0 + fb], in_=cs[:])

            def author_compute(blk, t):
                """ts (+DVE tt) for one block-step; returns accum closure."""
                c0, fb = OFFS[blk], BLOCKS[blk]
                thr = float(2.0 ** t)
                if t == 0:
                    u[blk] = u_pool.tile([P, fb], f32, tag=f"u{blk}",
                                         name=f"u{blk}")
                    # U_0 = xs_0 (v0 = 0); plain HWDGE load
                    nc.sync.dma_start(out=u[blk][:],
                                      in_=x_r[0][:, c0:c0 + fb])
                    w6[blk] = w_pool.tile([P, T * fb], f8, tag=f"w6b{blk}",
                                          name=f"w6_{blk}")
                sl = w6[blk][:, t * fb:(t + 1) * fb]
                if t == T - 1 and t5act:
                    # sigma = sign(U - 32) in {-1,0,1} on the idle ScalarE
                    nc.scalar.sign(out=sl, in_=u[blk][:], bias=biasm32[:])
                elif t == T - 1 and t5pool:
                    # s5 = (U >= 32) in {0,1} on the Pool engine; the PE
                    # applies the 32x weight via the ident32 stationary
                    nc.gpsimd.tensor_tensor(
                        out=sl, in0=u[blk][:],
                        in1=thr32[:].to_broadcast((P, fb)), op=A.is_ge)
                else:
                    # w = (U >= 2^t) * 2^t -> fp8e4 {0, 2^t}, both exact
                    nc.vector.tensor_scalar(
                        out=sl, in0=u[blk][:], scalar1=thr, scalar2=thr,
                        op0=A.is_ge, op1=A.mult,
                    )
                on_pool = (blk, t) in GPSIMD_STT
                if t < T - 1 and not on_pool:
                    # U = U - w, in place (tensor_tensor: valid on both
                    # DVE and Pool ISAs; scalar_tensor_tensor is DVE-only)
                    nc.vector.tensor_tensor(
                        out=u[blk][:], in0=u[blk][:], in1=sl,
                        op=A.subtract,
                    )
                return sl, on_pool

            def author_pool_tt(blk, t, sl):
                nc.gpsimd.tensor_tensor(
                    out=u[blk][:], in0=u[blk][:], in1=sl, op=A.subtract)

            def author_accum(blk, t):
                c0, fb = OFFS[blk], BLOCKS[blk]
                # U += xs_{t+1}: accumulate during load (SWDGE CCE add)
                nc.gpsimd.dma_start(
                    out=u[blk][:], in_=x_r[t + 1][:, c0:c0 + fb],
                    accum_op=A.add,
                )

            def author_pack(blk, t):
                c0, fb = OFFS[blk], BLOCKS[blk]
                last_blk = blk >= NBLK - nlast
                if last_blk:
                    # last block: pack per step into a dedicated PSUM
                    # accumulator so only t=5's matmuls are in the tail
                    if t == 0:
                        cp_last[blk] = cpl_pool.tile(
                            [P, max(BLOCKS[NBLK - nlast:])], f32,
                            tag="cpl", name=f"cpl{blk}")
                    for j0 in range(0, fb, MM):
                        m = min(MM, fb - j0)
                        o = t * fb + j0
                        if t == T - 1 and t5act:
                            nc.tensor.matmul(
                                cp_last[blk][:, j0:j0 + m],
                                ident32[:], w6[blk][:, o:o + m],
                                start=False, stop=False,
                            )
                            nc.tensor.matmul(
                                cp_last[blk][:, j0:j0 + m],
                                ident32[:], ones[:, j0:j0 + m],
                                start=False, stop=True,
                            )
                        elif t == T - 1 and t5pool:
                            nc.tensor.matmul(
                                cp_last[blk][:, j0:j0 + m],
                                ident32[:], w6[blk][:, o:o + m],
                                start=False, stop=True,
                            )
                        else:
                            nc.tensor.matmul(
                                cp_last[blk][:, j0:j0 + m],
                                ident[:], w6[blk][:, o:o + m],
                                start=(t == 0), stop=(t == T - 1),
                            )
                    if t == T - 1:
                        finish_block(blk, cp_last[blk][:, :fb])
                elif t == T - 1:
                    # other blocks: end-of-block PE burst
                    maxfb = max(BLOCKS[:-1])
                    cp_full = cp_pool.tile([P, maxfb], f32, tag="cp",
                                           name=f"cp{blk}")
                    cp = cp_full[:, :fb]
                    for j0 in range(0, fb, MM):
                        m = min(MM, fb - j0)
                        for tt in range(T):
                            o = tt * fb + j0
                            if tt == T - 1 and t5act:
                                nc.tensor.matmul(
                                    cp[:, j0:j0 + m], ident32[:],
                                    w6[blk][:, o:o + m],
                                    start=False, stop=False,
                                )
                                nc.tensor.matmul(
                                    cp[:, j0:j0 + m], ident32[:],
                                    ones[:, j0:j0 + m],
                                    start=False, stop=True,
                                )
                            elif tt == T - 1 and t5pool:
                                nc.tensor.matmul(
                                    cp[:, j0:j0 + m], ident32[:],
                                    w6[blk][:, o:o + m],
                                    start=False, stop=True,
                                )
                            else:
                                nc.tensor.matmul(
                                    cp[:, j0:j0 + m],
                                    ident[:],
                                    w6[blk][:, o:o + m],
                                    start=(tt == 0), stop=(tt == T - 1),
                                )
                    finish_block(blk, cp)

            # skewed rounds: block b runs step (r - SKEW*b); ops authored
            # inline per block (ts, tt, accum, pack)
            order = list(range(NBLK))
            if REV:
                order = order[::-1]
            for r in range(T + SKEW * (NBLK - 1)):
                for blk in order:
                    t = r - SKEW * blk
                    if not (0 <= t < T):
                        continue
                    sl, on_pool = author_compute(blk, t)
                    if t < T - 1:
                        if on_pool:
                            author_pool_tt(blk, t, sl)
                        author_accum(blk, t)
                    author_pack(blk, t)
    nc.finalize()
    return nc


_XS_SCALE = (2.0 ** np.arange(T, dtype=np.float32)).reshape(T, 1)


def kernel(input_current: np.ndarray) -> np.ndarray:
    global _COMPILED, LAST_RESULTS
    x = np.asarray(input_current, dtype=np.float32)
    assert x.shape == (T, B, C, H, W), x.shape

    if _COMPILED is None:
        _COMPILED = _build_program()
    nc = _COMPILED

    in_maps = []
    for k in range(N_CORES):
        shard = np.ascontiguousarray(
            x[:, k * B_PER_CORE:(k + 1) * B_PER_CORE]
        ).reshape(T, N_PER_CORE)
        # xs_t = 2^t * x_t (exact in fp32: pure exponent shift)
        in_maps.append({"x": shard * _XS_SCALE})

    trace = bool(int(os.environ.get("LIF_TRACE", "0")))
    res = run_bass_kernel_spmd(nc, in_maps, core_ids=list(range(N_CORES)),
                               trace=trace)
    LAST_RESULTS = res

    out = np.empty((T, B, C, H, W), dtype=np.float32)
    bits = np.arange(T, dtype=np.uint8).reshape(T, 1)
    for k in range(N_CORES):
        code = res.results[k]["c"]  # uint8 [N_PER_CORE]
        if T5ACT:
            # C = (bits 0..4) + 32*sigma + 32, contribution in {0,32,64}
            sp = np.empty((T, N_PER_CORE), dtype=np.float32)
            low = code & np.uint8(31)
            sp[:T - 1] = (low[None, :] >> bits[:T - 1]) & 1
            sp[T - 1] = code >= 32
        else:
            sp = ((code[None, :] >> bits) & 1).astype(np.float32)
        out[:, k * B_PER_CORE:(k + 1) * B_PER_CORE] = (
            sp.astype(np.float32).reshape(T, B_PER_CORE, C, H, W)
        )
    return out
